# revision 1
# baseline (speedup 1.0000x reference)
"""Trainium2 Bass kernel for nn_GCNModel (MMGCN/GCNII message passing).

Strategy (data-parallel over dialogues, 8 NeuronCores, no collectives):
  - Host: assign dialogues to cores (LPT), pad each core to a common
    utterance count U; gather/transpose per-core inputs; fold the GCNII
    theta/residual arithmetic into the 64 conv weights:
        h_{l+1} = relu([A@h, h0] @ W''_l),
        W''_l   = theta_l*W_l + [[c1_l*I],[c2_l*I]].
  - Device per core: projections -> block adjacency (arccos via
    2*atan(sqrt((1-y)/(1+y)))) -> sym-normalize -> 64 folded GCNII layers
    (bf16 matmuls, fp32 PSUM) -> head + log_softmax.
  - Host: scatter per-core rows back to the (411, 7) output.
"""
import os
import numpy as np
import ml_dtypes

import concourse.bass as bass
import concourse.mybir as mybir
import concourse.tile as tile
from concourse import bacc
from concourse.bass_utils import run_bass_kernel_spmd

NCORES = 8
H, G = 300, 500
NLAYERS = 64
LAMDA, ALPHA = 0.5, 0.1

BF = mybir.dt.bfloat16
F32 = mybir.dt.float32
AF = mybir.ActivationFunctionType
OP = mybir.AluOpType
AX = mybir.AxisListType

_BUILD_CACHE = {}


last_results = None  # BassKernelResults from the most recent kernel() call


def _chunks(total, size):
    return [(o, min(size, total - o)) for o in range(0, total, size)]


def _pad128(k):
    return ((k + 127) // 128) * 128


def _lpt_assign(lengths, n_bins):
    order = np.argsort(-np.asarray(lengths), kind="stable")
    bins = [[] for _ in range(n_bins)]
    loads = np.zeros(n_bins, dtype=np.int64)
    for d in order:
        b = int(np.argmin(loads))
        bins[b].append(int(d))
        loads[b] += lengths[d]
    return bins, loads


def _bf(x):
    return np.ascontiguousarray(np.asarray(x, np.float32).astype(ml_dtypes.bfloat16))


def _f32(x):
    return np.ascontiguousarray(np.asarray(x, np.float32))


def build_kernel(U, Ka, Kv, Kt, Kx):
    """Build the per-core SPMD Bass program. All K* are multiples of 128.

    Node layout: modality m's utterance u lives at row m*U_al + u, where
    U_al = ceil32(U). Rows [m*U_al+U, (m+1)*U_al) are dead padding kept at
    zero so every partition-offset access is 32-aligned.
    """
    U_al = ((U + 31) // 32) * 32
    R = 3 * U_al
    assert U <= 128, f"per-core utterance count {U} > 128 unsupported"
    assert R <= 512

    nc = bacc.Bacc("TRN2", target_bir_lowering=False, debug=False,
                   num_devices=NCORES)

    # ---- DRAM I/O ----
    fa_d = nc.dram_tensor("fa", [Ka, U], BF, kind="ExternalInput")
    fv_d = nc.dram_tensor("fv", [Kv, U], BF, kind="ExternalInput")
    ft_d = nc.dram_tensor("ft", [Kt, U], BF, kind="ExternalInput")
    mask_d = nc.dram_tensor("mask", [U, U], F32, kind="ExternalInput")
    Wa_d = nc.dram_tensor("Wa", [Ka, H], BF, kind="ExternalInput")
    Wv_d = nc.dram_tensor("Wv", [Kv, H], BF, kind="ExternalInput")
    Wt_d = nc.dram_tensor("Wt", [Kt, H], BF, kind="ExternalInput")
    Wx_d = nc.dram_tensor("Wx", [Kx, G], BF, kind="ExternalInput")
    Wc_d = nc.dram_tensor("Wc", [NLAYERS, 2 * G, G], BF, kind="ExternalInput")
    Wf_d = nc.dram_tensor("Wf", [3 * G, 7], BF, kind="ExternalInput")
    bf1_d = nc.dram_tensor("bf1", [1, 7], BF, kind="ExternalInput")
    idf_d = nc.dram_tensor("idf", [128, 128], F32, kind="ExternalInput")
    idb_d = nc.dram_tensor("idb", [128, 128], BF, kind="ExternalInput")
    out_d = nc.dram_tensor("out", [U, 7], F32, kind="ExternalOutput")

    rtiles = _chunks(R, 128)                # node-row tiles
    ftiles = _chunks(G, 128)                # feature tiles of 500
    nrt, nft = len(rtiles), len(ftiles)
    h300 = _chunks(H, 128)                  # projection output tiles {128,128,44}
    # ones row of xT: first 32-aligned row at/after feature H
    o_ti, o_tr = H // 128, ((H % 128) + 31) // 32 * 32
    if o_tr >= 128:
        o_ti, o_tr = o_ti + 1, 0
    ones_feat = o_ti * 128 + o_tr           # host puts b_in at this Wx row
    assert ones_feat < Kx
    # supportT k-chunks for the layer matmul: hiT tiles then h0T tiles
    # h0T (constant) half first so layer-l W-matmuls start before hiT copies land
    wkc = [(G + fo, fs) for fo, fs in ftiles] + [(fo, fs) for fo, fs in ftiles]

    def row_pieces(lo, ln):
        """Split node rows [lo, lo+ln) by rtile boundaries ->
        (rt_i, part_lo_within_tile, piece_len, offset_within_block)."""
        out = []
        done = 0
        while done < ln:
            g = lo + done
            rt_i = g // 128
            plo = g - rt_i * 128
            plen = min(128 - plo, ln - done)
            plen = min(plen, rtiles[rt_i][1] - plo)
            out.append((rt_i, plo, plen, done))
            done += plen
        return out

    with tile.TileContext(nc) as tc:
        with (
            tc.tile_pool(name="const", bufs=1) as cp,
            tc.tile_pool(name="state", bufs=3) as hp,
            tc.tile_pool(name="wc", bufs=8) as wp,
            tc.tile_pool(name="hiT", bufs=3) as ip,
            tc.tile_pool(name="psA", bufs=1, space="PSUM") as psA,
            tc.tile_pool(name="psO", bufs=2, space="PSUM") as psO,
        ):
            # ---- persistent SBUF ----
            A_sb = [cp.tile([rs, R], BF, tag=f"A{i}", name=f"A{i}")
                    for i, (ro, rs) in enumerate(rtiles)]
            h0T_sb = [cp.tile([fs, R], BF, tag=f"h0T{i}", name=f"h0T{i}")
                      for i, (fo, fs) in enumerate(ftiles)]
            nkx = Kx // 128
            xT_sb = [cp.tile([128, R], BF, tag=f"xT{i}", name=f"xT{i}")
                     for i in range(nkx)]
            ones_c = cp.tile([128, 1], F32, tag="ones_c", name="ones_c")
            idf_sb = cp.tile([128, 128], F32, tag="idf", name="idf_sb")
            idb_sb = cp.tile([128, 128], BF, tag="idb", name="idb_sb")
            mask_sb = cp.tile([U, U], F32, tag="mask", name="mask_sb")
            bf1_sb = cp.tile([1, 7], BF, tag="bf1", name="bf1_sb")
            ones_rb = cp.tile([1, 128], BF, tag="ones_rb", name="ones_rb")
            nc.vector.memset(ones_rb[:], 1.0)
            nc.vector.memset(ones_c[:], 1.0)
            nc.sync.dma_start(idf_sb[:], idf_d[:])
            nc.sync.dma_start(idb_sb[:], idb_d[:])
            nc.sync.dma_start(mask_sb[:], mask_d[:])
            nc.sync.dma_start(bf1_sb[:], bf1_d[:])

            h_tiles = [None] * nrt

            # ================= stage P/A/h0 (scoped) =================
            with tc.tile_pool(name="stg", bufs=1) as sp:
                for t in xT_sb:
                    nc.vector.memset(t[:, :R], 0.0)
                ones_m = sp.tile([128, 128], F32, tag="ones_m", name="ones_m")
                nc.vector.memset(ones_m[:], 1.0)

                # ---- projections, normal orientation: x_m = (fm^T Wm) [U,300]
                x_sb = []
                for m, (f_d, w_d, K) in enumerate(
                        [(fa_d, Wa_d, Ka), (fv_d, Wv_d, Kv), (ft_d, Wt_d, Kt)]):
                    kcs = _chunks(K, 128)
                    fsb = []
                    wsb = []
                    for ki, (ko, ks) in enumerate(kcs):
                        ftl = sp.tile([128, U], BF, tag=f"pf{m}_{ki}", name=f"pf{m}_{ki}")
                        nc.sync.dma_start(ftl[:ks, :], f_d[ko:ko + ks, :])
                        fsb.append(ftl)
                        wtl = sp.tile([128, H], BF, tag=f"pw{m}_{ki}", name=f"pw{m}_{ki}")
                        nc.sync.dma_start(wtl[:ks, :], w_d[ko:ko + ks, :])
                        wsb.append(wtl)
                    xp = psO.tile([U, H], F32, tag="psO0", name=f"xp{m}")
                    for ki, (ko, ks) in enumerate(kcs):
                        nc.tensor.matmul(xp[:U, :H], fsb[ki][:ks, :U],
                                         wsb[ki][:ks, :H],
                                         start=(ki == 0), stop=(ki == len(kcs) - 1))
                    xm = sp.tile([U, H], BF, tag=f"x{m}", name=f"x{m}")
                    nc.scalar.copy(xm[:U, :H], xp[:U, :H])
                    x_sb.append(xm)

                # ---- transpose x into xT (feature-major) ----
                for m in range(3):
                    c0 = m * U_al
                    for ki, (ko, ks) in enumerate(h300):
                        tpp = psO.tile([128, U], BF, tag="psO1", name=f"tx{m}_{ki}")
                        nc.tensor.transpose(tpp[:ks, :U], x_sb[m][:U, ko:ko + ks],
                                            idb_sb[:U, :U])
                        nc.scalar.copy(xT_sb[ki][:ks, c0:c0 + U], tpp[:ks, :U])
                # the ones row (feature index ones_feat), all R columns
                nc.vector.memset(xT_sb[o_ti][o_tr:o_tr + 1, :R], 1.0)

                # ---- h0 (normal, bf16 state) and h0T ----
                wxsb = [sp.tile([128, G], BF, tag=f"wx{i}", name=f"wx{i}")
                        for i in range(nkx)]
                for ki in range(nkx):
                    nc.sync.dma_start(wxsb[ki][:, :], Wx_d[ki * 128:(ki + 1) * 128, :])
                for rt_i, (ro, rs) in enumerate(rtiles):
                    pso = psO.tile([rs, G], F32, tag=f"psO{rt_i}", name=f"h0p{rt_i}")
                    for ki in range(nkx):
                        nc.tensor.matmul(pso[:rs, :G], xT_sb[ki][:, ro:ro + rs],
                                         wxsb[ki][:, :G],
                                         start=(ki == 0), stop=(ki == nkx - 1))
                    ht = hp.tile([rs, G], BF, tag=f"h{rt_i}", name=f"h0_{rt_i}")
                    nc.scalar.activation(ht[:rs, :G], pso[:rs, :G], AF.Relu)
                    h_tiles[rt_i] = ht
                for ft_i, (fo, fs) in enumerate(ftiles):
                    psa = psA.tile([fs, R], F32, tag=f"psA{ft_i}", name=f"h0Tp{ft_i}")
                    for ki in range(nkx):
                        nc.tensor.matmul(psa[:fs, :R], wxsb[ki][:, fo:fo + fs],
                                         xT_sb[ki][:, :R],
                                         start=(ki == 0), stop=(ki == nkx - 1))
                    nc.scalar.activation(h0T_sb[ft_i][:fs, :R], psa[:fs, :R], AF.Relu)


                # ---- norms and cross dots via accum_out: one DVE op each ----
                sqdum = sp.tile([U, H], F32, tag="sqdum", name="sqdum")
                acc6 = sp.tile([U, 8], F32, tag="acc6", name="acc6")
                pairs = [(0, 0), (1, 1), (2, 2), (0, 1), (0, 2), (1, 2)]
                for k, (m, n) in enumerate(pairs):
                    nc.vector.scalar_tensor_tensor(
                        sqdum[:U, :H], x_sb[m][:U, :H], 1.0, x_sb[n][:U, :H],
                        op0=OP.mult, op1=OP.mult, accum_out=acc6[:U, k:k + 1])
                # inv3 = 1/(sqrt(nsq)+1e-8)
                inv3 = sp.tile([U, 3], F32, tag="inv3", name="inv3")
                nc.scalar.activation(inv3[:U, :3], acc6[:U, :3], AF.Sqrt)
                nc.vector.tensor_scalar_add(inv3[:U, :3], inv3[:U, :3], 1e-8)
                nc.vector.reciprocal(inv3[:U, :3], inv3[:U, :3])

                # ---- intra-modal gram + two-sided inv scaling -> yw [U, 3U]
                yw = sp.tile([U, 3 * U], F32, tag="yw", name="yw")
                t1 = sp.tile([U, U], F32, tag="t1", bufs=2, name="t1")
                for m in range(3):
                    c0 = m * U_al
                    gp = psO.tile([U, U], F32, tag="psO0", name=f"G{m}")
                    for ki, (ko, ks) in enumerate(h300):
                        xs = xT_sb[ki][:ks, c0:c0 + U]
                        nc.tensor.matmul(gp[:U, :U], xs, xs,
                                         start=(ki == 0), stop=(ki == len(h300) - 1))
                    nc.vector.tensor_scalar(t1[:U, :U], gp[:U, :U],
                                            inv3[:U, m:m + 1], None, op0=OP.mult)
                    t1t = psO.tile([U, U], F32, tag="psO1", name=f"t1t{m}")
                    nc.tensor.transpose(t1t[:U, :U], t1[:U, :U], idf_sb[:U, :U])
                    nc.vector.tensor_scalar(yw[:U, m * U:(m + 1) * U], t1t[:U, :U],
                                            inv3[:U, m:m + 1], None, op0=OP.mult)
                # cross: yc[:, k] = e * inv_m * inv_n
                yc = sp.tile([U, 4], F32, tag="yc", name="yc")
                for k, (m, n) in enumerate([(0, 1), (0, 2), (1, 2)]):
                    nc.vector.tensor_scalar(yc[:U, k:k + 1], acc6[:U, 3 + k:4 + k],
                                            inv3[:U, m:m + 1], inv3[:U, n:n + 1],
                                            op0=OP.mult, op1=OP.mult)

                # ---- clip + batched arccos similarity ----
                def clip_pre(t, p, n):
                    nc.vector.tensor_scalar(t[:p, :n], t[:p, :n], 0.99999, 1.0,
                                            op0=OP.mult, op1=OP.min)
                    nc.vector.tensor_scalar(t[:p, :n], t[:p, :n], -1.0, None,
                                            op0=OP.max)

                clip_pre(yw, U, 3 * U)
                clip_pre(yc, U, 3)
                denw = sp.tile([U, 3 * U], F32, tag="denw", name="denw")
                denc = sp.tile([U, 4], F32, tag="denc", name="denc")
                for y_, den_, n_ in [(yw, denw, 3 * U), (yc, denc, 3)]:
                    nc.vector.tensor_scalar(den_[:U, :n_], y_[:U, :n_], 1.0, 1e-6,
                                            op0=OP.add, op1=OP.max)
                    nc.vector.reciprocal(den_[:U, :n_], den_[:U, :n_])
                    nc.vector.tensor_scalar(y_[:U, :n_], y_[:U, :n_], -1.0, 1.0,
                                            op0=OP.mult, op1=OP.add)
                    nc.vector.tensor_mul(y_[:U, :n_], y_[:U, :n_], den_[:U, :n_])
                nc.scalar.activation(yw[:U, :3 * U], yw[:U, :3 * U], AF.Sqrt)
                nc.scalar.activation(yc[:U, :3], yc[:U, :3], AF.Sqrt)
                nc.scalar.activation(yw[:U, :3 * U], yw[:U, :3 * U], AF.Arctan)
                nc.scalar.activation(yc[:U, :3], yc[:U, :3], AF.Arctan)
                nc.vector.tensor_scalar(yw[:U, :3 * U], yw[:U, :3 * U],
                                        -2.0 / np.pi, 1.0, op0=OP.mult, op1=OP.add)
                nc.vector.tensor_scalar(yc[:U, :3], yc[:U, :3],
                                        -2.0 / np.pi, 1.0, op0=OP.mult, op1=OP.add)

                # ---- assemble Abig ----
                Ab_sb = [sp.tile([rs, R], F32, tag=f"Ab{i}", name=f"Ab{i}")
                         for i, (ro, rs) in enumerate(rtiles)]
                for rt_i, (ro, rs) in enumerate(rtiles):
                    nc.vector.memset(Ab_sb[rt_i][:rs, :R], 0.0)
                for m in range(3):
                    c0 = m * U_al
                    for (rt_i, plo, plen, boff) in row_pieces(c0, U):
                        nc.vector.tensor_mul(
                            Ab_sb[rt_i][plo:plo + plen, c0:c0 + U],
                            yw[boff:boff + plen, m * U:(m + 1) * U],
                            mask_sb[boff:boff + plen, :U])
                dful = sp.tile([U, U], F32, tag="dful", bufs=2, name="dful")
                for k, (m, n) in enumerate([(0, 1), (0, 2), (1, 2)]):
                    nc.vector.tensor_scalar(dful[:U, :U], ones_m[:U, :U],
                                            yc[:U, k:k + 1], None, op0=OP.mult)
                    for (bm, bn) in [(m, n), (n, m)]:
                        for (rt_i, plo, plen, boff) in row_pieces(bm * U_al, U):
                            nc.vector.tensor_mul(
                                Ab_sb[rt_i][plo:plo + plen,
                                            bn * U_al:bn * U_al + U],
                                dful[boff:boff + plen, :U],
                                idf_sb[boff:boff + plen, :U])

                # ---- degree + symmetric normalize -> A (bf16) ----
                degp = psA.tile([1, R], F32, tag="psA3", name="degp")
                for rt_i, (ro, rs) in enumerate(rtiles):
                    nc.tensor.matmul(degp[:1, :R], ones_c[:rs, :1],
                                     Ab_sb[rt_i][:rs, :R],
                                     start=(rt_i == 0), stop=(rt_i == nrt - 1))
                dsb = sp.tile([1, R], F32, tag="dsb", name="dsb")
                nc.vector.tensor_scalar(dsb[:1, :R], degp[:1, :R], 1e-12, None,
                                        op0=OP.max)
                sqd = sp.tile([1, R], F32, tag="sqd", name="sqd")
                nc.scalar.activation(sqd[:1, :R], dsb[:1, :R], AF.Sqrt)
                dinvT = sp.tile([1, R], F32, tag="dinvT", name="dinvT")
                nc.vector.reciprocal(dinvT[:1, :R], sqd[:1, :R])
                for rt_i, (ro, rs) in enumerate(rtiles):
                    op_ = psO.tile([128, R], F32, tag="psO1", name=f"O{rt_i}")
                    nc.tensor.matmul(op_[:rs, :R], dinvT[:1, ro:ro + rs],
                                     dinvT[:1, :R], start=True, stop=True)
                    nc.vector.tensor_mul(A_sb[rt_i][:rs, :R],
                                         Ab_sb[rt_i][:rs, :R], op_[:rs, :R])

            # ================= 64 GCNII layers =================
            n_layers = int(os.environ.get("BASS_GCN_LAYERS", str(NLAYERS)))
            for l in range(n_layers):
                w_sb = []
                for ki, (ko, ks) in enumerate(wkc):
                    wt = wp.tile([ks, G], BF, tag=f"wc{ki}", name=f"w{l}_{ki}")
                    nc.sync.dma_start(wt[:ks, :], Wc_d[l, ko:ko + ks, :])
                    w_sb.append(wt)
                psa_t = []
                for ft_i, (fo, fs) in enumerate(ftiles):
                    psa_t.append(psA.tile([fs, R], F32, tag=f"psA{ft_i}",
                                          name=f"hiTp{l}_{ft_i}"))
                for rt_i, (ro, rs) in enumerate(rtiles):
                    for ft_i, (fo, fs) in enumerate(ftiles):
                        nc.tensor.matmul(psa_t[ft_i][:fs, :R],
                                         h_tiles[rt_i][:rs, fo:fo + fs],
                                         A_sb[rt_i][:rs, :R],
                                         start=(rt_i == 0), stop=(rt_i == nrt - 1))
                hiT_sb = []
                for ft_i, (fo, fs) in enumerate(ftiles):
                    ht = ip.tile([fs, R], BF, tag=f"hiT{ft_i}", name=f"hiT{l}_{ft_i}")
                    nc.vector.tensor_copy(ht[:fs, :R], psa_t[ft_i][:fs, :R])
                    hiT_sb.append(ht)
                sup = h0T_sb + hiT_sb
                for rt_i, (ro, rs) in enumerate(rtiles):
                    pso = psO.tile([rs, G], F32, tag=f"psO{rt_i}", name=f"op{l}_{rt_i}")
                    for ki in range(len(wkc)):
                        ksz = ftiles[ki % nft][1]
                        nc.tensor.matmul(pso[:rs, :G],
                                         sup[ki][:ksz, ro:ro + rs],
                                         w_sb[ki][:ksz, :G],
                                         start=(ki == 0), stop=(ki == len(wkc) - 1))
                    nh = hp.tile([rs, G], BF, tag=f"h{rt_i}", name=f"h{l}_{rt_i}")
                    nc.scalar.activation(nh[:rs, :G], pso[:rs, :G], AF.Relu)
                    h_tiles[rt_i] = nh

            # ================= head =================
            with tc.tile_pool(name="hd", bufs=1) as hd:
                lg = psA.tile([7, U], F32, tag="psA0", name="lg")
                ki = 0
                for m in range(3):
                    pieces = row_pieces(m * U_al, U)
                    direct = (len(pieces) == 1 and pieces[0][1] in (0, 32, 64))
                    if direct:
                        rt_i, plo, _, _ = pieces[0]
                        hm = h_tiles[rt_i][plo:plo + U, :G]
                        idd = idb_sb[plo:plo + U, plo:plo + U]
                    else:
                        hmt = hd.tile([U, G], BF, tag="hm", bufs=2, name=f"hm{m}")
                        for (rt_i, plo, plen, boff) in pieces:
                            nc.vector.tensor_copy(hmt[boff:boff + plen, :G],
                                                  h_tiles[rt_i][plo:plo + plen, :G])
                        hm = hmt
                        idd = idb_sb[:U, :U]
                    for ft_i, (fo, fs) in enumerate(ftiles):
                        tp = psO.tile([fs, U], BF, tag="psO0", name=f"tp{m}_{ft_i}")
                        nc.tensor.transpose(tp[:fs, :U], hm[:U, fo:fo + fs],
                                            idd)
                        fT = hd.tile([fs, U], BF, tag="fT", bufs=2, name=f"fT{m}_{ft_i}")
                        nc.scalar.activation(fT[:fs, :U], tp[:fs, :U], AF.Relu)
                        wfs = hd.tile([fs, 7], BF, tag="wfs", bufs=2, name=f"wf{m}_{ft_i}")
                        nc.sync.dma_start(wfs[:fs, :], Wf_d[m * G + fo:m * G + fo + fs, :])
                        nc.tensor.matmul(lg[:7, :U], wfs[:fs, :7], fT[:fs, :U],
                                         start=(ki == 0), stop=False)
                        ki += 1
                nc.tensor.matmul(lg[:7, :U], bf1_sb[:1, :7], ones_rb[:1, :U],
                                 start=False, stop=True)
                lgs = hd.tile([7, U], F32, tag="lgs", name="lgs")
                nc.vector.tensor_copy(lgs[:7, :U], lg[:7, :U])
                lt = psA.tile([U, 7], F32, tag="psA1", name="lt")
                nc.tensor.transpose(lt[:U, :7], lgs[:7, :U], idf_sb[:7, :7])
                nmx = hd.tile([U, 1], F32, tag="nmx", name="nmx")
                nc.vector.reduce_max(nmx[:U, :1], lt[:U, :7], AX.X, negate=True)
                esum = hd.tile([U, 1], F32, tag="esum", name="esum")
                edum = hd.tile([U, 7], F32, tag="edum", name="edum")
                nc.scalar.activation(edum[:U, :7], lt[:U, :7], AF.Exp,
                                     bias=nmx[:U, :1], accum_out=esum[:U, :1])
                nls = hd.tile([U, 1], F32, tag="nls", name="nls")
                nc.scalar.activation(nls[:U, :1], esum[:U, :1], AF.Ln)
                nc.vector.tensor_scalar_mul(nls[:U, :1], nls[:U, :1], -1.0)
                osb = hd.tile([U, 7], F32, tag="osb", name="osb")
                nc.vector.tensor_scalar(osb[:U, :7], lt[:U, :7], nmx[:U, :1],
                                        nls[:U, :1], op0=OP.add, op1=OP.add)
                nc.sync.dma_start(out_d[:, :], osb[:U, :7])

    nc.compile()
    nc._gcn_ones_feat = ones_feat
    return nc


def _prep_shared(inputs, Ka, Kv, Kt, Kx, spk):
    """Host-side shared (replicated) weight arrays."""
    Wa, ba = inputs["Wa"], inputs["ba"]
    Wv, bv = inputs["Wv"], inputs["bv"]
    Wt, bt = inputs["Wt"], inputs["bt"]
    spk_emb = inputs["spk_emb"]
    W_in, b_in = inputs["W_in"], inputs["b_in"]
    W_convs = inputs["W_convs"]
    W_fc1, b_fc1 = inputs["W_fc1"], inputs["b_fc1"]

    def padK(a, K):
        out = np.zeros((K, a.shape[1]), np.float32)
        out[:a.shape[0]] = a
        return out

    Wa_aug = padK(np.concatenate([_f32(Wa), _f32(ba)[None, :]], 0), Ka)
    Wv_aug = padK(np.concatenate([_f32(Wv), _f32(bv)[None, :]], 0), Kv)
    Wt_aug = padK(np.concatenate([_f32(Wt), _f32(bt)[None, :], _f32(spk_emb)], 0), Kt)
    o_ti, o_tr = H // 128, ((H % 128) + 31) // 32 * 32
    if o_tr >= 128:
        o_ti, o_tr = o_ti + 1, 0
    ones_feat = o_ti * 128 + o_tr
    Wx_aug = np.zeros((Kx, G), np.float32)
    Wx_aug[:H] = _f32(W_in)
    Wx_aug[ones_feat] = _f32(b_in)

    ls = np.arange(1, NLAYERS + 1, dtype=np.float64)
    theta = np.log(LAMDA / ls + 1.0)
    c1 = (1.0 - theta) * (1.0 - ALPHA)
    c2 = (1.0 - theta) * ALPHA
    Wfold = theta[:, None, None] * np.asarray(W_convs, np.float64)
    idx = np.arange(G)
    for l in range(NLAYERS):
        Wfold[l, idx, idx] += c1[l]
        Wfold[l, G + idx, idx] += c2[l]

    iden = np.eye(128, dtype=np.float32)
    return {
        "Wa": _bf(Wa_aug), "Wv": _bf(Wv_aug), "Wt": _bf(Wt_aug),
        "Wx": _bf(Wx_aug), "Wc": _bf(Wfold),
        "Wf": _bf(W_fc1), "bf1": _bf(_f32(b_fc1).reshape(1, 7)),
        "idf": _f32(iden), "idb": _bf(iden),
    }


def kernel(**inputs):
    global last_results
    inputs = {k: np.asarray(v) for k, v in inputs.items()}
    seq_idx = inputs["seq_idx"].astype(np.int64)
    batch_idx = inputs["batch_idx"].astype(np.int64)
    dia_id = inputs["dia_id"].astype(np.int64)
    fea_a, fea_v, fea_t = inputs["fea_a"], inputs["fea_v"], inputs["fea_t"]
    speaker = inputs["speaker"]
    spk_emb = inputs["spk_emb"]
    N = seq_idx.shape[0]
    NSPK = spk_emb.shape[0]

    # ---- shard dialogues over cores ----
    uniq, counts = np.unique(dia_id, return_counts=True)
    bins, loads = _lpt_assign(counts, NCORES)
    U = max(int(loads.max()), 1)
    positions = {int(d): np.where(dia_id == d)[0] for d in uniq}
    core_utts = []
    for b in range(NCORES):
        if bins[b]:
            idx = np.sort(np.concatenate([positions[d] for d in bins[b]]))
        else:
            idx = np.zeros(0, np.int64)
        core_utts.append(idx.astype(np.int64))

    Ka = _pad128(fea_a.shape[2] + 1)
    Kv = _pad128(fea_v.shape[2] + 1)
    Kt = _pad128(fea_t.shape[2] + 1 + NSPK)
    Kx = _pad128(H + 1)

    spk = np.argmax(_f32(speaker)[seq_idx, batch_idx], axis=-1)

    shared = _prep_shared(inputs, Ka, Kv, Kt, Kx, spk)

    in_maps = []
    for b in range(NCORES):
        utts = core_utts[b]
        nreal = len(utts)
        fa = np.zeros((Ka, U), np.float32)
        fv = np.zeros((Kv, U), np.float32)
        ft = np.zeros((Kt, U), np.float32)
        mask = np.zeros((U, U), np.float32)
        if nreal:
            fa[:fea_a.shape[2], :nreal] = _f32(fea_a)[seq_idx[utts], batch_idx[utts]].T
            fa[fea_a.shape[2], :nreal] = 1.0
            fv[:fea_v.shape[2], :nreal] = _f32(fea_v)[seq_idx[utts], batch_idx[utts]].T
            fv[fea_v.shape[2], :nreal] = 1.0
            dt = fea_t.shape[2]
            ft[:dt, :nreal] = _f32(fea_t)[seq_idx[utts], batch_idx[utts]].T
            ft[dt, :nreal] = 1.0
            oh = np.zeros((NSPK, nreal), np.float32)
            oh[spk[utts], np.arange(nreal)] = 1.0
            ft[dt + 1:dt + 1 + NSPK, :nreal] = oh
            dd = dia_id[utts]
            mask[:nreal, :nreal] = (dd[:, None] == dd[None, :]).astype(np.float32)
        in_maps.append({
            "fa": _bf(fa), "fv": _bf(fv), "ft": _bf(ft), "mask": mask,
            **shared,
        })

    key = (U, Ka, Kv, Kt, Kx)
    if key not in _BUILD_CACHE:
        _BUILD_CACHE[key] = build_kernel(*key)
    nc = _BUILD_CACHE[key]

    trace = bool(int(os.environ.get("BASS_GCN_TRACE", "0")))
    res = run_bass_kernel_spmd(nc, in_maps, core_ids=list(range(NCORES)),
                               trace=trace)
    last_results = res

    out_full = np.zeros((N, 7), np.float32)
    for b in range(NCORES):
        utts = core_utts[b]
        if len(utts):
            out_full[utts] = np.asarray(res.results[b]["out"], np.float32)[:len(utts)]
    return out_full



# revision 8
# speedup vs baseline: 1.2518x; 1.2518x over previous
"""Trainium2 Bass kernel for nn_GCNModel (MMGCN/GCNII message passing).

Strategy (data-parallel over dialogues, 8 NeuronCores, no collectives):
  - Host: assign dialogues to cores (LPT), pad each core to a common
    utterance count U; gather/transpose per-core inputs; fold the GCNII
    theta/residual arithmetic into the 64 conv weights:
        h_{l+1} = relu([A@h, h0] @ W''_l),
        W''_l   = theta_l*W_l + [[c1_l*I],[c2_l*I]].
  - Device per core: projections -> block adjacency (arccos via
    2*atan(sqrt((1-y)/(1+y)))) -> sym-normalize -> 64 folded GCNII layers
    (bf16 matmuls, fp32 PSUM) -> head + log_softmax.
  - Host: scatter per-core rows back to the (411, 7) output.
"""
import os
import numpy as np
import ml_dtypes

import concourse.bass as bass
import concourse.mybir as mybir
import concourse.tile as tile
from concourse import bacc
from concourse.bass_utils import run_bass_kernel_spmd

NCORES = 8
H, G = 300, 500
NLAYERS = 64
LAMDA, ALPHA = 0.5, 0.1

BF = mybir.dt.bfloat16
F32 = mybir.dt.float32
AF = mybir.ActivationFunctionType
OP = mybir.AluOpType
AX = mybir.AxisListType

_BUILD_CACHE = {}


last_results = None  # BassKernelResults from the most recent kernel() call


def _chunks(total, size):
    return [(o, min(size, total - o)) for o in range(0, total, size)]


def _pad128(k):
    return ((k + 127) // 128) * 128


def _lpt_assign(lengths, n_bins):
    order = np.argsort(-np.asarray(lengths), kind="stable")
    bins = [[] for _ in range(n_bins)]
    loads = np.zeros(n_bins, dtype=np.int64)
    for d in order:
        b = int(np.argmin(loads))
        bins[b].append(int(d))
        loads[b] += lengths[d]
    return bins, loads


def _bf(x):
    return np.ascontiguousarray(np.asarray(x, np.float32).astype(ml_dtypes.bfloat16))


def _f32(x):
    return np.ascontiguousarray(np.asarray(x, np.float32))


def build_kernel(U, Ka, Kv, Kt, Kx):
    """Build the per-core SPMD Bass program. All K* are multiples of 128.

    Node layout: modality m's utterance u lives at row m*U_al + u, where
    U_al = ceil32(U). Rows [m*U_al+U, (m+1)*U_al) are dead padding kept at
    zero so every partition-offset access is 32-aligned.
    """
    U_al = ((U + 31) // 32) * 32
    R = 3 * U_al
    assert U <= 128, f"per-core utterance count {U} > 128 unsupported"
    assert R <= 512

    nc = bacc.Bacc("TRN2", target_bir_lowering=False, debug=False,
                   num_devices=NCORES)

    # ---- DRAM I/O ----
    fa_d = nc.dram_tensor("fa", [Ka, U], BF, kind="ExternalInput")
    fv_d = nc.dram_tensor("fv", [Kv, U], BF, kind="ExternalInput")
    ft_d = nc.dram_tensor("ft", [Kt, U], BF, kind="ExternalInput")
    mask_d = nc.dram_tensor("mask", [U, U], F32, kind="ExternalInput")
    Wa_d = nc.dram_tensor("Wa", [Ka, H], BF, kind="ExternalInput")
    Wv_d = nc.dram_tensor("Wv", [Kv, H], BF, kind="ExternalInput")
    Wt_d = nc.dram_tensor("Wt", [Kt, H], BF, kind="ExternalInput")
    Wx_d = nc.dram_tensor("Wx", [Kx, G], BF, kind="ExternalInput")
    # per-layer weights pre-packed on host: 8 k-chunks (h0T chunks then hiT
    # chunks, each zero-padded to 128 rows) side by side -> one DMA per layer
    Wc_d = nc.dram_tensor("Wc", [NLAYERS, 128, 8 * G], BF, kind="ExternalInput")
    Wf_d = nc.dram_tensor("Wf", [3 * G, 7], BF, kind="ExternalInput")
    bf1_d = nc.dram_tensor("bf1", [1, 7], BF, kind="ExternalInput")
    idf_d = nc.dram_tensor("idf", [128, 128], F32, kind="ExternalInput")
    idb_d = nc.dram_tensor("idb", [128, 128], BF, kind="ExternalInput")
    out_d = nc.dram_tensor("out", [U, 7], F32, kind="ExternalOutput")

    rtiles = _chunks(R, 128)                # node-row tiles
    ftiles = _chunks(G, 128)                # feature tiles of 500
    nrt, nft = len(rtiles), len(ftiles)
    h300 = _chunks(H, 128)                  # projection output tiles {128,128,44}
    # ones row of xT: first 32-aligned row at/after feature H
    o_ti, o_tr = H // 128, ((H % 128) + 31) // 32 * 32
    if o_tr >= 128:
        o_ti, o_tr = o_ti + 1, 0
    ones_feat = o_ti * 128 + o_tr           # host puts b_in at this Wx row
    assert ones_feat < Kx
    # supportT k-chunks for the layer matmul: hiT tiles then h0T tiles
    # h0T (constant) half first so layer-l W-matmuls start before hiT copies land
    wkc = [(G + fo, fs) for fo, fs in ftiles] + [(fo, fs) for fo, fs in ftiles]

    def row_pieces(lo, ln):
        """Split node rows [lo, lo+ln) by rtile boundaries ->
        (rt_i, part_lo_within_tile, piece_len, offset_within_block)."""
        out = []
        done = 0
        while done < ln:
            g = lo + done
            rt_i = g // 128
            plo = g - rt_i * 128
            plen = min(128 - plo, ln - done)
            plen = min(plen, rtiles[rt_i][1] - plo)
            out.append((rt_i, plo, plen, done))
            done += plen
        return out

    with tile.TileContext(nc) as tc:
        with (
            tc.tile_pool(name="const", bufs=1) as cp,
            tc.tile_pool(name="state", bufs=3) as hp,
            tc.tile_pool(name="wc", bufs=3) as wp,
            tc.tile_pool(name="hiT", bufs=3) as ip,
            tc.tile_pool(name="psA", bufs=1, space="PSUM") as psA,
            tc.tile_pool(name="psO", bufs=2, space="PSUM") as psO,
        ):
            # ---- persistent SBUF ----
            A_sb = [cp.tile([rs, R], BF, tag=f"A{i}", name=f"A{i}")
                    for i, (ro, rs) in enumerate(rtiles)]
            h0T_sb = [cp.tile([fs, R], BF, tag=f"h0T{i}", name=f"h0T{i}")
                      for i, (fo, fs) in enumerate(ftiles)]
            nkx = Kx // 128
            xT_sb = [cp.tile([128, R], BF, tag=f"xT{i}", name=f"xT{i}")
                     for i in range(nkx)]
            ones_c = cp.tile([128, 1], F32, tag="ones_c", name="ones_c")
            idf_sb = cp.tile([128, 128], F32, tag="idf", name="idf_sb")
            idb_sb = cp.tile([128, 128], BF, tag="idb", name="idb_sb")
            mask_sb = cp.tile([U, U], F32, tag="mask", name="mask_sb")
            bf1_sb = cp.tile([1, 7], BF, tag="bf1", name="bf1_sb")
            ones_rb = cp.tile([1, 128], BF, tag="ones_rb", name="ones_rb")
            nc.vector.memset(ones_rb[:], 1.0)
            nc.vector.memset(ones_c[:], 1.0)
            nc.sync.dma_start(idf_sb[:], idf_d[:])
            nc.sync.dma_start(idb_sb[:], idb_d[:])
            nc.sync.dma_start(mask_sb[:], mask_d[:])
            nc.sync.dma_start(bf1_sb[:], bf1_d[:])

            h_tiles = [None] * nrt

            # ================= stage P/A/h0 (scoped) =================
            with tc.tile_pool(name="stg", bufs=1) as sp:
                for t in xT_sb:
                    nc.vector.memset(t[:, :R], 0.0)
                ones_m = sp.tile([128, 128], F32, tag="ones_m", name="ones_m")
                nc.vector.memset(ones_m[:], 1.0)

                # ---- projections, normal orientation: x_m = (fm^T Wm) [U,300]
                x_sb = []
                for m, (f_d, w_d, K) in enumerate(
                        [(fa_d, Wa_d, Ka), (fv_d, Wv_d, Kv), (ft_d, Wt_d, Kt)]):
                    kcs = _chunks(K, 128)
                    fsb = []
                    wsb = []
                    for ki, (ko, ks) in enumerate(kcs):
                        ftl = sp.tile([128, U], BF, tag=f"pf{m}_{ki}", name=f"pf{m}_{ki}")
                        nc.sync.dma_start(ftl[:ks, :], f_d[ko:ko + ks, :])
                        fsb.append(ftl)
                        wtl = sp.tile([128, H], BF, tag=f"pw{m}_{ki}", name=f"pw{m}_{ki}")
                        nc.sync.dma_start(wtl[:ks, :], w_d[ko:ko + ks, :])
                        wsb.append(wtl)
                    xp = psO.tile([U, H], F32, tag="psO0", name=f"xp{m}")
                    for ki, (ko, ks) in enumerate(kcs):
                        nc.tensor.matmul(xp[:U, :H], fsb[ki][:ks, :U],
                                         wsb[ki][:ks, :H],
                                         start=(ki == 0), stop=(ki == len(kcs) - 1))
                    xm = sp.tile([U, H], BF, tag=f"x{m}", name=f"x{m}")
                    nc.scalar.copy(xm[:U, :H], xp[:U, :H])
                    x_sb.append(xm)

                # ---- transpose x into xT (feature-major) ----
                for m in range(3):
                    c0 = m * U_al
                    for ki, (ko, ks) in enumerate(h300):
                        tpp = psO.tile([128, U], BF, tag="psO1", name=f"tx{m}_{ki}")
                        nc.tensor.transpose(tpp[:ks, :U], x_sb[m][:U, ko:ko + ks],
                                            idb_sb[:U, :U])
                        nc.scalar.copy(xT_sb[ki][:ks, c0:c0 + U], tpp[:ks, :U])
                # the ones row (feature index ones_feat), all R columns
                nc.vector.memset(xT_sb[o_ti][o_tr:o_tr + 1, :R], 1.0)

                # ---- h0 (normal, bf16 state) and h0T ----
                wxsb = [sp.tile([128, G], BF, tag=f"wx{i}", name=f"wx{i}")
                        for i in range(nkx)]
                for ki in range(nkx):
                    nc.sync.dma_start(wxsb[ki][:, :], Wx_d[ki * 128:(ki + 1) * 128, :])
                for rt_i, (ro, rs) in enumerate(rtiles):
                    pso = psO.tile([rs, G], F32, tag=f"psO{rt_i}", name=f"h0p{rt_i}")
                    for ki in range(nkx):
                        nc.tensor.matmul(pso[:rs, :G], xT_sb[ki][:, ro:ro + rs],
                                         wxsb[ki][:, :G],
                                         start=(ki == 0), stop=(ki == nkx - 1))
                    ht = hp.tile([rs, G], BF, tag=f"h{rt_i}", name=f"h0_{rt_i}")
                    nc.scalar.activation(ht[:rs, :G], pso[:rs, :G], AF.Relu)
                    h_tiles[rt_i] = ht
                for ft_i, (fo, fs) in enumerate(ftiles):
                    psa = psA.tile([fs, R], F32, tag=f"psA{ft_i}", name=f"h0Tp{ft_i}")
                    for ki in range(nkx):
                        nc.tensor.matmul(psa[:fs, :R], wxsb[ki][:, fo:fo + fs],
                                         xT_sb[ki][:, :R],
                                         start=(ki == 0), stop=(ki == nkx - 1))
                    nc.scalar.activation(h0T_sb[ft_i][:fs, :R], psa[:fs, :R], AF.Relu)


                # ---- norms and cross dots via accum_out: one DVE op each ----
                sqdum = sp.tile([U, H], F32, tag="sqdum", name="sqdum")
                acc6 = sp.tile([U, 8], F32, tag="acc6", name="acc6")
                pairs = [(0, 0), (1, 1), (2, 2), (0, 1), (0, 2), (1, 2)]
                for k, (m, n) in enumerate(pairs):
                    nc.vector.scalar_tensor_tensor(
                        sqdum[:U, :H], x_sb[m][:U, :H], 1.0, x_sb[n][:U, :H],
                        op0=OP.mult, op1=OP.mult, accum_out=acc6[:U, k:k + 1])
                # inv3 = 1/(sqrt(nsq)+1e-8)
                inv3 = sp.tile([U, 3], F32, tag="inv3", name="inv3")
                nc.scalar.activation(inv3[:U, :3], acc6[:U, :3], AF.Sqrt)
                nc.vector.tensor_scalar_add(inv3[:U, :3], inv3[:U, :3], 1e-8)
                nc.vector.reciprocal(inv3[:U, :3], inv3[:U, :3])

                # ---- intra-modal gram + two-sided inv scaling -> yw [U, 3U]
                yw = sp.tile([U, 3 * U], F32, tag="yw", name="yw")
                t1 = sp.tile([U, U], F32, tag="t1", bufs=2, name="t1")
                for m in range(3):
                    c0 = m * U_al
                    gp = psO.tile([U, U], F32, tag="psO0", name=f"G{m}")
                    for ki, (ko, ks) in enumerate(h300):
                        xs = xT_sb[ki][:ks, c0:c0 + U]
                        nc.tensor.matmul(gp[:U, :U], xs, xs,
                                         start=(ki == 0), stop=(ki == len(h300) - 1))
                    nc.vector.tensor_scalar(t1[:U, :U], gp[:U, :U],
                                            inv3[:U, m:m + 1], None, op0=OP.mult)
                    t1t = psO.tile([U, U], F32, tag="psO1", name=f"t1t{m}")
                    nc.tensor.transpose(t1t[:U, :U], t1[:U, :U], idf_sb[:U, :U])
                    nc.vector.tensor_scalar(yw[:U, m * U:(m + 1) * U], t1t[:U, :U],
                                            inv3[:U, m:m + 1], None, op0=OP.mult)
                # cross: yc[:, k] = e * inv_m * inv_n
                yc = sp.tile([U, 4], F32, tag="yc", name="yc")
                for k, (m, n) in enumerate([(0, 1), (0, 2), (1, 2)]):
                    nc.vector.tensor_scalar(yc[:U, k:k + 1], acc6[:U, 3 + k:4 + k],
                                            inv3[:U, m:m + 1], inv3[:U, n:n + 1],
                                            op0=OP.mult, op1=OP.mult)

                # ---- clip + batched arccos similarity ----
                def clip_pre(t, p, n):
                    nc.vector.tensor_scalar(t[:p, :n], t[:p, :n], 0.99999, 1.0,
                                            op0=OP.mult, op1=OP.min)
                    nc.vector.tensor_scalar(t[:p, :n], t[:p, :n], -1.0, None,
                                            op0=OP.max)

                clip_pre(yw, U, 3 * U)
                clip_pre(yc, U, 3)
                denw = sp.tile([U, 3 * U], F32, tag="denw", name="denw")
                denc = sp.tile([U, 4], F32, tag="denc", name="denc")
                for y_, den_, n_ in [(yw, denw, 3 * U), (yc, denc, 3)]:
                    nc.vector.tensor_scalar(den_[:U, :n_], y_[:U, :n_], 1.0, 1e-6,
                                            op0=OP.add, op1=OP.max)
                    nc.vector.reciprocal(den_[:U, :n_], den_[:U, :n_])
                    nc.vector.tensor_scalar(y_[:U, :n_], y_[:U, :n_], -1.0, 1.0,
                                            op0=OP.mult, op1=OP.add)
                    nc.vector.tensor_mul(y_[:U, :n_], y_[:U, :n_], den_[:U, :n_])
                nc.scalar.activation(yw[:U, :3 * U], yw[:U, :3 * U], AF.Sqrt)
                nc.scalar.activation(yc[:U, :3], yc[:U, :3], AF.Sqrt)
                nc.scalar.activation(yw[:U, :3 * U], yw[:U, :3 * U], AF.Arctan)
                nc.scalar.activation(yc[:U, :3], yc[:U, :3], AF.Arctan)
                nc.vector.tensor_scalar(yw[:U, :3 * U], yw[:U, :3 * U],
                                        -2.0 / np.pi, 1.0, op0=OP.mult, op1=OP.add)
                nc.vector.tensor_scalar(yc[:U, :3], yc[:U, :3],
                                        -2.0 / np.pi, 1.0, op0=OP.mult, op1=OP.add)

                # ---- assemble Abig ----
                Ab_sb = [sp.tile([rs, R], F32, tag=f"Ab{i}", name=f"Ab{i}")
                         for i, (ro, rs) in enumerate(rtiles)]
                for rt_i, (ro, rs) in enumerate(rtiles):
                    nc.vector.memset(Ab_sb[rt_i][:rs, :R], 0.0)
                for m in range(3):
                    c0 = m * U_al
                    for (rt_i, plo, plen, boff) in row_pieces(c0, U):
                        nc.vector.tensor_mul(
                            Ab_sb[rt_i][plo:plo + plen, c0:c0 + U],
                            yw[boff:boff + plen, m * U:(m + 1) * U],
                            mask_sb[boff:boff + plen, :U])
                dful = sp.tile([U, U], F32, tag="dful", bufs=2, name="dful")
                for k, (m, n) in enumerate([(0, 1), (0, 2), (1, 2)]):
                    nc.vector.tensor_scalar(dful[:U, :U], ones_m[:U, :U],
                                            yc[:U, k:k + 1], None, op0=OP.mult)
                    for (bm, bn) in [(m, n), (n, m)]:
                        for (rt_i, plo, plen, boff) in row_pieces(bm * U_al, U):
                            nc.vector.tensor_mul(
                                Ab_sb[rt_i][plo:plo + plen,
                                            bn * U_al:bn * U_al + U],
                                dful[boff:boff + plen, :U],
                                idf_sb[boff:boff + plen, :U])

                # ---- degree + symmetric normalize -> A (bf16) ----
                degp = psA.tile([1, R], F32, tag="psA3", name="degp")
                for rt_i, (ro, rs) in enumerate(rtiles):
                    nc.tensor.matmul(degp[:1, :R], ones_c[:rs, :1],
                                     Ab_sb[rt_i][:rs, :R],
                                     start=(rt_i == 0), stop=(rt_i == nrt - 1))
                dsb = sp.tile([1, R], F32, tag="dsb", name="dsb")
                nc.vector.tensor_scalar(dsb[:1, :R], degp[:1, :R], 1e-12, None,
                                        op0=OP.max)
                sqd = sp.tile([1, R], F32, tag="sqd", name="sqd")
                nc.scalar.activation(sqd[:1, :R], dsb[:1, :R], AF.Sqrt)
                dinvT = sp.tile([1, R], F32, tag="dinvT", name="dinvT")
                nc.vector.reciprocal(dinvT[:1, :R], sqd[:1, :R])
                for rt_i, (ro, rs) in enumerate(rtiles):
                    op_ = psO.tile([128, R], F32, tag="psO1", name=f"O{rt_i}")
                    nc.tensor.matmul(op_[:rs, :R], dinvT[:1, ro:ro + rs],
                                     dinvT[:1, :R], start=True, stop=True)
                    nc.vector.tensor_mul(A_sb[rt_i][:rs, :R],
                                         Ab_sb[rt_i][:rs, :R], op_[:rs, :R])

            # ================= 64 GCNII layers =================
            n_layers = int(os.environ.get("BASS_GCN_LAYERS", str(NLAYERS)))
            for l in range(n_layers):
                wt = wp.tile([128, 8 * G], BF, tag="wc", name=f"w{l}")
                nc.sync.dma_start(wt[:, :], Wc_d[l, :, :])
                psa_t = []
                for ft_i, (fo, fs) in enumerate(ftiles):
                    psa_t.append(psA.tile([fs, R], F32, tag=f"psA{ft_i}",
                                          name=f"hiTp{l}_{ft_i}"))
                for rt_i, (ro, rs) in enumerate(rtiles):
                    for ft_i, (fo, fs) in enumerate(ftiles):
                        nc.tensor.matmul(psa_t[ft_i][:fs, :R],
                                         h_tiles[rt_i][:rs, fo:fo + fs],
                                         A_sb[rt_i][:rs, :R],
                                         start=(rt_i == 0), stop=(rt_i == nrt - 1))
                hiT_sb = []
                for ft_i, (fo, fs) in enumerate(ftiles):
                    ht = ip.tile([fs, R], BF, tag=f"hiT{ft_i}", name=f"hiT{l}_{ft_i}")
                    nc.vector.tensor_copy(ht[:fs, :R], psa_t[ft_i][:fs, :R])
                    hiT_sb.append(ht)
                sup = h0T_sb + hiT_sb
                for rt_i, (ro, rs) in enumerate(rtiles):
                    pso = psO.tile([rs, G], F32, tag=f"psO{rt_i}", name=f"op{l}_{rt_i}")
                    for ki in range(len(wkc)):
                        ksz = ftiles[ki % nft][1]
                        nc.tensor.matmul(pso[:rs, :G],
                                         sup[ki][:ksz, ro:ro + rs],
                                         wt[:ksz, ki * G:(ki + 1) * G],
                                         start=(ki == 0), stop=(ki == len(wkc) - 1))
                    nh = hp.tile([rs, G], BF, tag=f"h{rt_i}", name=f"h{l}_{rt_i}")
                    nc.scalar.activation(nh[:rs, :G], pso[:rs, :G], AF.Relu)
                    h_tiles[rt_i] = nh

            # ================= head =================
            with tc.tile_pool(name="hd", bufs=1) as hd:
                lg = psA.tile([7, U], F32, tag="psA0", name="lg")
                ki = 0
                for m in range(3):
                    pieces = row_pieces(m * U_al, U)
                    direct = (len(pieces) == 1 and pieces[0][1] in (0, 32, 64))
                    if direct:
                        rt_i, plo, _, _ = pieces[0]
                        hm = h_tiles[rt_i][plo:plo + U, :G]
                        idd = idb_sb[plo:plo + U, plo:plo + U]
                    else:
                        hmt = hd.tile([U, G], BF, tag="hm", bufs=2, name=f"hm{m}")
                        for (rt_i, plo, plen, boff) in pieces:
                            nc.vector.tensor_copy(hmt[boff:boff + plen, :G],
                                                  h_tiles[rt_i][plo:plo + plen, :G])
                        hm = hmt
                        idd = idb_sb[:U, :U]
                    for ft_i, (fo, fs) in enumerate(ftiles):
                        tp = psO.tile([fs, U], BF, tag="psO0", name=f"tp{m}_{ft_i}")
                        nc.tensor.transpose(tp[:fs, :U], hm[:U, fo:fo + fs],
                                            idd)
                        fT = hd.tile([fs, U], BF, tag="fT", bufs=2, name=f"fT{m}_{ft_i}")
                        nc.scalar.activation(fT[:fs, :U], tp[:fs, :U], AF.Relu)
                        wfs = hd.tile([fs, 7], BF, tag="wfs", bufs=2, name=f"wf{m}_{ft_i}")
                        nc.sync.dma_start(wfs[:fs, :], Wf_d[m * G + fo:m * G + fo + fs, :])
                        nc.tensor.matmul(lg[:7, :U], wfs[:fs, :7], fT[:fs, :U],
                                         start=(ki == 0), stop=False)
                        ki += 1
                nc.tensor.matmul(lg[:7, :U], bf1_sb[:1, :7], ones_rb[:1, :U],
                                 start=False, stop=True)
                lgs = hd.tile([7, U], F32, tag="lgs", name="lgs")
                nc.vector.tensor_copy(lgs[:7, :U], lg[:7, :U])
                lt = psA.tile([U, 7], F32, tag="psA1", name="lt")
                nc.tensor.transpose(lt[:U, :7], lgs[:7, :U], idf_sb[:7, :7])
                nmx = hd.tile([U, 1], F32, tag="nmx", name="nmx")
                nc.vector.reduce_max(nmx[:U, :1], lt[:U, :7], AX.X, negate=True)
                esum = hd.tile([U, 1], F32, tag="esum", name="esum")
                edum = hd.tile([U, 7], F32, tag="edum", name="edum")
                nc.scalar.activation(edum[:U, :7], lt[:U, :7], AF.Exp,
                                     bias=nmx[:U, :1], accum_out=esum[:U, :1])
                nls = hd.tile([U, 1], F32, tag="nls", name="nls")
                nc.scalar.activation(nls[:U, :1], esum[:U, :1], AF.Ln)
                nc.vector.tensor_scalar_mul(nls[:U, :1], nls[:U, :1], -1.0)
                osb = hd.tile([U, 7], F32, tag="osb", name="osb")
                nc.vector.tensor_scalar(osb[:U, :7], lt[:U, :7], nmx[:U, :1],
                                        nls[:U, :1], op0=OP.add, op1=OP.add)
                nc.sync.dma_start(out_d[:, :], osb[:U, :7])

    nc.compile()
    nc._gcn_ones_feat = ones_feat
    return nc


def _prep_shared(inputs, Ka, Kv, Kt, Kx, spk):
    """Host-side shared (replicated) weight arrays."""
    Wa, ba = inputs["Wa"], inputs["ba"]
    Wv, bv = inputs["Wv"], inputs["bv"]
    Wt, bt = inputs["Wt"], inputs["bt"]
    spk_emb = inputs["spk_emb"]
    W_in, b_in = inputs["W_in"], inputs["b_in"]
    W_convs = inputs["W_convs"]
    W_fc1, b_fc1 = inputs["W_fc1"], inputs["b_fc1"]

    def padK(a, K):
        out = np.zeros((K, a.shape[1]), np.float32)
        out[:a.shape[0]] = a
        return out

    Wa_aug = padK(np.concatenate([_f32(Wa), _f32(ba)[None, :]], 0), Ka)
    Wv_aug = padK(np.concatenate([_f32(Wv), _f32(bv)[None, :]], 0), Kv)
    Wt_aug = padK(np.concatenate([_f32(Wt), _f32(bt)[None, :], _f32(spk_emb)], 0), Kt)
    o_ti, o_tr = H // 128, ((H % 128) + 31) // 32 * 32
    if o_tr >= 128:
        o_ti, o_tr = o_ti + 1, 0
    ones_feat = o_ti * 128 + o_tr
    Wx_aug = np.zeros((Kx, G), np.float32)
    Wx_aug[:H] = _f32(W_in)
    Wx_aug[ones_feat] = _f32(b_in)

    ls = np.arange(1, NLAYERS + 1, dtype=np.float64)
    theta = np.log(LAMDA / ls + 1.0)
    c1 = (1.0 - theta) * (1.0 - ALPHA)
    c2 = (1.0 - theta) * ALPHA
    Wfold = theta[:, None, None] * np.asarray(W_convs, np.float64)
    idx = np.arange(G)
    for l in range(NLAYERS):
        Wfold[l, idx, idx] += c1[l]
        Wfold[l, G + idx, idx] += c2[l]

    # repack into [NLAYERS, 128, 8*G]: 8 k-chunks (h0T tiles then hiT tiles,
    # matching build_kernel's wkc order), each zero-padded to 128 rows
    ftiles = _chunks(G, 128)
    wkc = [(G + fo, fs) for fo, fs in ftiles] + [(fo, fs) for fo, fs in ftiles]
    Wpack = np.zeros((NLAYERS, 128, 8 * G), np.float64)
    for ki, (ko, ks) in enumerate(wkc):
        Wpack[:, :ks, ki * G:(ki + 1) * G] = Wfold[:, ko:ko + ks, :]

    iden = np.eye(128, dtype=np.float32)
    return {
        "Wa": _bf(Wa_aug), "Wv": _bf(Wv_aug), "Wt": _bf(Wt_aug),
        "Wx": _bf(Wx_aug), "Wc": _bf(Wpack),
        "Wf": _bf(W_fc1), "bf1": _bf(_f32(b_fc1).reshape(1, 7)),
        "idf": _f32(iden), "idb": _bf(iden),
    }


def kernel(**inputs):
    global last_results
    inputs = {k: np.asarray(v) for k, v in inputs.items()}
    seq_idx = inputs["seq_idx"].astype(np.int64)
    batch_idx = inputs["batch_idx"].astype(np.int64)
    dia_id = inputs["dia_id"].astype(np.int64)
    fea_a, fea_v, fea_t = inputs["fea_a"], inputs["fea_v"], inputs["fea_t"]
    speaker = inputs["speaker"]
    spk_emb = inputs["spk_emb"]
    N = seq_idx.shape[0]
    NSPK = spk_emb.shape[0]

    # ---- shard dialogues over cores ----
    uniq, counts = np.unique(dia_id, return_counts=True)
    bins, loads = _lpt_assign(counts, NCORES)
    U = max(int(loads.max()), 1)
    positions = {int(d): np.where(dia_id == d)[0] for d in uniq}
    core_utts = []
    for b in range(NCORES):
        if bins[b]:
            idx = np.sort(np.concatenate([positions[d] for d in bins[b]]))
        else:
            idx = np.zeros(0, np.int64)
        core_utts.append(idx.astype(np.int64))

    Ka = _pad128(fea_a.shape[2] + 1)
    Kv = _pad128(fea_v.shape[2] + 1)
    Kt = _pad128(fea_t.shape[2] + 1 + NSPK)
    Kx = _pad128(H + 1)

    spk = np.argmax(_f32(speaker)[seq_idx, batch_idx], axis=-1)

    shared = _prep_shared(inputs, Ka, Kv, Kt, Kx, spk)

    in_maps = []
    for b in range(NCORES):
        utts = core_utts[b]
        nreal = len(utts)
        fa = np.zeros((Ka, U), np.float32)
        fv = np.zeros((Kv, U), np.float32)
        ft = np.zeros((Kt, U), np.float32)
        mask = np.zeros((U, U), np.float32)
        if nreal:
            fa[:fea_a.shape[2], :nreal] = _f32(fea_a)[seq_idx[utts], batch_idx[utts]].T
            fa[fea_a.shape[2], :nreal] = 1.0
            fv[:fea_v.shape[2], :nreal] = _f32(fea_v)[seq_idx[utts], batch_idx[utts]].T
            fv[fea_v.shape[2], :nreal] = 1.0
            dt = fea_t.shape[2]
            ft[:dt, :nreal] = _f32(fea_t)[seq_idx[utts], batch_idx[utts]].T
            ft[dt, :nreal] = 1.0
            oh = np.zeros((NSPK, nreal), np.float32)
            oh[spk[utts], np.arange(nreal)] = 1.0
            ft[dt + 1:dt + 1 + NSPK, :nreal] = oh
            dd = dia_id[utts]
            mask[:nreal, :nreal] = (dd[:, None] == dd[None, :]).astype(np.float32)
        in_maps.append({
            "fa": _bf(fa), "fv": _bf(fv), "ft": _bf(ft), "mask": mask,
            **shared,
        })

    key = (U, Ka, Kv, Kt, Kx)
    if key not in _BUILD_CACHE:
        _BUILD_CACHE[key] = build_kernel(*key)
    nc = _BUILD_CACHE[key]

    trace = bool(int(os.environ.get("BASS_GCN_TRACE", "0")))
    res = run_bass_kernel_spmd(nc, in_maps, core_ids=list(range(NCORES)),
                               trace=trace)
    last_results = res

    out_full = np.zeros((N, 7), np.float32)
    for b in range(NCORES):
        utts = core_utts[b]
        if len(utts):
            out_full[utts] = np.asarray(res.results[b]["out"], np.float32)[:len(utts)]
    return out_full



# revision 31
# speedup vs baseline: 1.7189x; 1.3731x over previous
"""Trainium2 Bass kernel for nn_GCNModel (MMGCN/GCNII message passing).

Strategy (data-parallel over dialogues, 8 NeuronCores, no collectives):
  Host: LPT-assign dialogues to cores (max load U); gather/pack per-core
  inputs; per-layer conv weights are quantized to fp8-e4m3 as s_l*theta_l*W_l
  (s_l a power of two) and packed for DoubleRow matmuls.

  Device layer loop keeps the GCNII state in BOTH orientations:
    hT   [500, R] feature-major (bf16)  -- R = 3U node rows
    h_rm [R, 500] row-major (bf16), pre-scaled by P_l = S*s_l*c1_l
  Per layer (R=3U<=256, G=500, 4 g-tiles of 500, 2 row-slots of R):
    stage1:  psA[g] = h_rm.T-contract with A      (bf16, = P_l * hiT)
    cast:    hiT_f8[g] = psA[g] / (s_l*c1_l)      (fp8, = S*hiT)
    c2mm:    psA[g] += (S*s_l*c2_l * I) @ h0T_bf  (bf16 identity matmul)
    stage2': psA[g] += Wq_l.T-pairs @ [h0T_f8; hiT_f8]  (fp8 DoubleRow,
             carries S*s_l*theta_l*(sup @ W))
    relu:    hT[g] = relu(psA[g] / (S*s_l))
    transpose+copy: h_rm = hT.T * P_{l+1}
  All scale bookkeeping is exact (powers of two and fp32 immediates); the
  residual path stays bf16/fp32 so fp8 only touches the theta*(sup@W) term.

  Head: logits.T accumulated from hT directly, log-softmax, scatter on host.
"""
import os
import numpy as np
import ml_dtypes

import concourse.bass as bass
import concourse.mybir as mybir
import concourse.tile as tile
from concourse import bacc
from concourse.bass_utils import run_bass_kernel_spmd

NCORES = 8
H, G = 300, 500
NLAYERS = 64
LAMDA, ALPHA = 0.5, 0.1
S_SUP = 64.0          # fixed power-of-two scale for fp8 support activations

BF = mybir.dt.bfloat16
F32 = mybir.dt.float32
F8 = mybir.dt.float8e4
AF = mybir.ActivationFunctionType
OP = mybir.AluOpType
AX = mybir.AxisListType
DR = mybir.MatmulPerfMode.DoubleRow

_BUILD_CACHE = {}

last_results = None  # BassKernelResults from the most recent kernel() call


def _chunks(total, size):
    return [(o, min(size, total - o)) for o in range(0, total, size)]


def _pad128(k):
    return ((k + 127) // 128) * 128


def _lpt_assign(lengths, n_bins):
    order = np.argsort(-np.asarray(lengths), kind="stable")
    bins = [[] for _ in range(n_bins)]
    loads = np.zeros(n_bins, dtype=np.int64)
    for d in order:
        b = int(np.argmin(loads))
        bins[b].append(int(d))
        loads[b] += lengths[d]
    return bins, loads


def _bf(x):
    return np.ascontiguousarray(np.asarray(x, np.float32).astype(ml_dtypes.bfloat16))


def _f32(x):
    return np.ascontiguousarray(np.asarray(x, np.float32))


def _layer_consts():
    ls = np.arange(1, NLAYERS + 1, dtype=np.float64)
    theta = np.log(LAMDA / ls + 1.0)
    c1 = (1.0 - theta) * (1.0 - ALPHA)
    c2 = (1.0 - theta) * ALPHA
    return theta, c1, c2



def transpose_tail(nc, ps, srcT, h_rm, idb, gtiles, slots, R, P, uid):
    """Transpose feature-major srcT back to row-major h_rm, scaled by P.

    Uses 4 half PSUM banks (per slot x per column-half) so the H1 copies
    never anti-depend on the g2/g3 transposes, and next-layer stage1 g0/g1
    can start as soon as cols 0:256 land.
    """
    G = 500
    tags = ["pa0", "pa1", "po0", "po1"]
    pth = [ps.tile([128, 1024], BF, tag=tags[i], bufs=2, name=f"pth{uid}_{i}")
           for i in range(4)]
    for g, (go, gsz) in enumerate(gtiles):
        half, hco = (0, go) if g < 2 else (1, go - 256)
        for si, (so, ssz) in enumerate(slots):
            nc.tensor.matmul(pth[si * 2 + half][:ssz, hco:hco + gsz],
                             srcT[:gsz, g, so:so + ssz],
                             idb[:gsz, :gsz], is_transpose=True,
                             start=(g % 2 == 0), stop=(g % 2 == 1),
                             skip_group_check=True)
        if g == 1:
            for si, (so, ssz) in enumerate(slots):
                if si == 0:
                    nc.vector.tensor_scalar(h_rm[:ssz, si, 0:256],
                                            pth[0][:ssz, 0:256],
                                            P, None, op0=OP.mult)
                else:
                    nc.scalar.mul(h_rm[:ssz, si, 0:256], pth[2][:ssz, 0:256], P)
    nc.vector.tensor_scalar(h_rm[:128, 0, 256:G], pth[1][:128, 0:G - 256],
                            P, None, op0=OP.mult)
    s1 = slots[1][1]
    nc.scalar.mul(h_rm[:s1, 1, 256:G], pth[3][:s1, 0:G - 256], P)


def build_kernel(U, nka, nkv, nkt):
    """Per-core SPMD Bass program.

    U: utterances per core (padded); R = 3U node rows as 2 slots (128, R-128).
    nka/nkv/nkt: number of 128-row k-chunks for the a/v/t projections.
    """
    U_al = ((U + 31) // 32) * 32        # 32-aligned modality stride
    R = 3 * U_al
    assert 128 < R <= 256, f"R={R} out of supported range"
    nkf = nka + nkv + nkt
    slots = [(0, 128), (128, R - 128)]       # node-row slots
    gtiles = [(i * 128, 128) for i in range(4)]  # G padded to 512
    htiles = _chunks(H, 128)                 # 3 projection h-slices
    theta, c1, c2 = _layer_consts()
    # per-layer pow2 weight scales, computed from the actual W absmax on host;
    # passed in via module-level _W_SCALES set by kernel() before build.
    s_l = build_kernel._w_scales
    n_layers = int(os.environ.get("BASS_GCN_LAYERS", str(NLAYERS)))

    nc = bacc.Bacc("TRN2", target_bir_lowering=False, debug=False,
                   num_devices=NCORES)

    # ---- DRAM I/O (all host-packed, partition-major) ----
    f_d = nc.dram_tensor("f_all", [128, nkf, U], BF, kind="ExternalInput")
    wp_d = nc.dram_tensor("Wproj", [128, nkf, H], BF, kind="ExternalInput")
    wx_d = nc.dram_tensor("Wxp", [128, 3, 512], BF, kind="ExternalInput")
    wq_d = nc.dram_tensor("Wq", [NLAYERS, 128, 4 * 2 * G], F8, kind="ExternalInput")
    cb_d = nc.dram_tensor("cbf", [128, 219], BF, kind="ExternalInput")
    cf_d = nc.dram_tensor("cf32", [128, U + 128], F32, kind="ExternalInput")
    out_d = nc.dram_tensor("out", [U, 7], F32, kind="ExternalOutput")

    def row_pieces(lo, ln):
        """Split node rows [lo, lo+ln) by slot boundary ->
        (slot, part_lo, piece_len, offset_within_block)."""
        out = []
        done = 0
        while done < ln:
            r = lo + done
            si = r // 128
            plo = r - si * 128
            plen = min(ln - done, slots[si][1] - plo)
            out.append((si, plo, plen, done))
            done += plen
        return out

    with tile.TileContext(nc) as tc:
        with (
            tc.tile_pool(name="const", bufs=1) as cp,
            tc.tile_pool(name="wc", bufs=3) as wp,
            tc.tile_pool(name="ps", bufs=1, space="PSUM") as ps,
        ):
            # ---- persistent SBUF ----
            A2 = cp.tile([128, 2, R], BF, tag="A2", name="A2")
            xT = cp.tile([128, 3, R], BF, tag="xT", name="xT")
            h0T_bf = cp.tile([128, 4, R], BF, tag="h0T_bf", name="h0T_bf")
            h0T_f8 = cp.tile([128, 4, R], F8, tag="h0T_f8", name="h0T_f8")
            hiT_f8 = cp.tile([128, 4, R], F8, tag="hiT_f8", name="hiT_f8")
            h0_rm = cp.tile([128, 2, 512], BF, tag="h0_rm", name="h0_rm")
            h_rm = cp.tile([128, 2, 512], BF, tag="h_rm", name="h_rm", bufs=2)
            c2id = cp.tile([128, 128], BF, tag="c2id", name="c2id", bufs=2)
            cbf = cp.tile([128, 219], BF, tag="cbf", name="cbf")
            cf32 = cp.tile([128, U + 128], F32, tag="cf32", name="cf32")
            ones_c = cp.tile([128, 1], BF, tag="ones_c", name="ones_c")
            ones_r = cp.tile([1, 128], BF, tag="ones_r", name="ones_r")
            nc.vector.memset(ones_c[:], 1.0)
            nc.vector.memset(ones_r[:], 1.0)
            nc.vector.memset(A2[:, :, :], 0.0)

            idb = cbf[:, 0:128]              # bf16 identity
            idf = cf32[:, U:U + 128]         # f32 identity
            mask = cf32[:, 0:U]              # same-dialogue mask [U, U]

            nc.sync.dma_start(cbf[:, :], cb_d[:, :])
            nc.sync.dma_start(cf32[:, :], cf_d[:, :])

            # ================= preamble (scoped) =================
            with tc.tile_pool(name="stg", bufs=1) as sp:
                fsb = sp.tile([128, nkf, U], BF, tag="fsb", name="fsb")
                wpsb = sp.tile([128, nkf, H], BF, tag="wpsb", name="wpsb")
                wxsb = sp.tile([128, 3, 512], BF, tag="wxsb", name="wxsb")
                mk = [list(range(0, nka)), list(range(nka, nka + nkv)),
                      list(range(nka + nkv, nkf))]
                for m in range(3):      # per-modality DMAs so proj-m starts early
                    lo, hi_ = mk[m][0], mk[m][-1] + 1
                    nc.sync.dma_start(wpsb[:, lo:hi_, :], wp_d[:, lo:hi_, :])
                    nc.sync.dma_start(fsb[:, lo:hi_, :], f_d[:, lo:hi_, :])
                nc.sync.dma_start(wxsb[:, :, :], wx_d[:, :, :])
                for _si in range(2):
                    nc.vector.memset(h_rm[:, _si, 500:512], 0.0)
                    nc.vector.memset(h0_rm[:, _si, 500:512], 0.0)
                nc.vector.memset(xT[:, :, :], 0.0)
                nc.vector.memset(xT[64:65, 2, :R], 1.0)

                # ---- projections into xT (m-major, gram pipelined) ----
                acc = sp.tile([U, 8], F32, tag="acc", name="acc")
                sqd = sp.tile([U, U], F32, tag="sqd", name="sqd")
                gsb = [sp.tile([U, U], F32, tag=f"g{m}", name=f"g{m}")
                       for m in range(3)]
                pairs = [(0, 0), (1, 1), (2, 2), (0, 1), (0, 2), (1, 2)]
                pxt = ["pa0", "pa1", "po0"]
                px = [ps.tile([128, 512], F32, tag=pxt[hs], bufs=2,
                      name=f"px{hs}") for hs in range(3)]
                ceng = [nc.vector, None, nc.vector]
                gfired = set()

                def gram(k, m, n):
                    pg = ps.tile([128, 512], F32, tag="po1", bufs=2,
                                 name=f"pg{k}")
                    for hs, (ho, hsz) in enumerate(htiles):
                        hv = hsz if hs < 2 else H - 256   # real dims only
                        nc.tensor.matmul(pg[:U, :U],
                                         xT[:hv, hs, m * U_al:m * U_al + U],
                                         xT[:hv, hs, n * U_al:n * U_al + U],
                                         start=(hs == 0), stop=(hs == 2))
                    nc.vector.scalar_tensor_tensor(
                        sqd[:U, :U], pg[:U, :U], 1.0, idf[:U, :U],
                        op0=OP.mult, op1=OP.mult, accum_out=acc[:U, k:k + 1])
                    if m == n:
                        nc.scalar.copy(gsb[m][:U, :U], pg[:U, :U])

                for m in range(3):
                    for kc in mk[m]:
                        for hs, (ho, hsz) in enumerate(htiles):
                            nc.tensor.matmul(
                                px[hs][:hsz, m * U:(m + 1) * U],
                                wpsb[:, kc, ho:ho + hsz],
                                fsb[:, kc, :U],
                                start=(m == 0 and kc == mk[0][0]),
                                stop=(m == 2 and kc == mk[2][-1]),
                                skip_group_check=True)
                    for hs, (ho, hsz) in enumerate(htiles):
                        if hs == 1:
                            nc.scalar.mul(xT[:hsz, hs, m * U_al:m * U_al + U],
                                          px[hs][:hsz, m * U:(m + 1) * U], 1.0)
                        else:
                            ceng[hs].tensor_scalar(
                                xT[:hsz, hs, m * U_al:m * U_al + U],
                                px[hs][:hsz, m * U:(m + 1) * U],
                                1.0, None, op0=OP.mult)
                    # grams whose operands are now complete
                    for k, (gm, gn) in enumerate(pairs):
                        if k not in gfired and gm <= m and gn <= m:
                            gram(k, gm, gn)
                            gfired.add(k)

                # ---- h0T matmuls early (independent of A build) ----
                pa_b = []
                for b in range(2):
                    pa = ps.tile([128, 2, 192], F32, tag=f"pa{b}", bufs=2,
                                 name=f"h0p{b}")
                    for j in range(2):
                        g = 2 * b + j
                        go, gsz = gtiles[g]
                        for hs in range(3):
                            ksz = 128 if hs < 2 else 65   # 44 dims + ones @64
                            nc.tensor.matmul(pa[:gsz, j, :R],
                                             wxsb[:ksz, hs, go:go + gsz],
                                             xT[:ksz, hs, :R],
                                             start=(j == 0 and hs == 0),
                                             stop=(j == 1 and hs == 2),
                                             skip_group_check=True)
                    pa_b.append(pa)
                for b in range(2):
                    nc.scalar.activation(h0T_bf[:, 2 * b:2 * b + 2, :R],
                                         pa_b[b][:, :, :R], AF.Relu)
                    nc.vector.tensor_scalar(h0T_f8[:, 2 * b:2 * b + 2, :R],
                                            pa_b[b][:, :, :R], 0.0,
                                            float(S_SUP), op0=OP.max,
                                            op1=OP.mult)

                # ---- inverse norms, similarity chain ----
                inv3 = sp.tile([U, 3], F32, tag="inv3", name="inv3")
                nc.scalar.activation(inv3[:U, :3], acc[:U, :3], AF.Sqrt)
                nc.vector.tensor_scalar_add(inv3[:U, :3], inv3[:U, :3], 1e-8)
                nc.vector.reciprocal(inv3[:U, :3], inv3[:U, :3])

                yw = sp.tile([U, 3 * U + 3], F32, tag="yw", name="yw")
                t1 = sp.tile([U, U], F32, tag="t1", name="t1", bufs=2)
                for m in range(3):
                    nc.vector.tensor_scalar(t1[:U, :U], gsb[m][:U, :U],
                                            inv3[:U, m:m + 1], None, op0=OP.mult)
                    tp = ps.tile([128, 512], F32, tag="pa0", bufs=2,
                                 name=f"tp{m}")
                    nc.tensor.transpose(tp[:U, :U], t1[:U, :U], idf[:U, :U])
                    nc.vector.tensor_scalar(yw[:U, m * U:(m + 1) * U], tp[:U, :U],
                                            inv3[:U, m:m + 1], None, op0=OP.mult)
                for k, (m, n) in enumerate(pairs[3:]):
                    nc.vector.tensor_scalar(yw[:U, 3 * U + k:3 * U + k + 1],
                                            acc[:U, 3 + k:4 + k],
                                            inv3[:U, m:m + 1], inv3[:U, n:n + 1],
                                            op0=OP.mult, op1=OP.mult)

                # arccos(y)/pi via 2/pi*atan(sqrt((1-y)/(1+y))): sim = 1 - that
                W3 = 3 * U + 3
                den = sp.tile([U, W3], F32, tag="den", name="den")
                nc.vector.tensor_scalar(yw[:U, :W3], yw[:U, :W3], 0.99999, 1.0,
                                        op0=OP.mult, op1=OP.min)
                nc.vector.tensor_scalar(yw[:U, :W3], yw[:U, :W3], -1.0, None,
                                        op0=OP.max)
                nc.vector.tensor_scalar(den[:U, :W3], yw[:U, :W3], 1.0, 1e-6,
                                        op0=OP.add, op1=OP.max)
                nc.vector.reciprocal(den[:U, :W3], den[:U, :W3])
                nc.vector.tensor_scalar(yw[:U, :W3], yw[:U, :W3], -1.0, 1.0,
                                        op0=OP.mult, op1=OP.add)
                nc.vector.tensor_mul(yw[:U, :W3], yw[:U, :W3], den[:U, :W3])
                nc.scalar.activation(yw[:U, :W3], yw[:U, :W3], AF.Sqrt)
                nc.scalar.activation(yw[:U, :W3], yw[:U, :W3], AF.Arctan)
                nc.vector.tensor_scalar(yw[:U, :W3], yw[:U, :W3],
                                        -2.0 / np.pi, 1.0, op0=OP.mult, op1=OP.add)

                # ---- assemble A2 [128, 2, R] ----
                for m in range(3):
                    for (si, plo, plen, boff) in row_pieces(m * U_al, U):
                        nc.vector.tensor_mul(
                            A2[plo:plo + plen, si, m * U_al:m * U_al + U],
                            yw[boff:boff + plen, m * U:(m + 1) * U],
                            mask[boff:boff + plen, :U])
                asm_i = 0
                for k, (m, n) in enumerate(pairs[3:]):
                    for (bm, bn) in [(m, n), (n, m)]:
                        for (si, plo, plen, boff) in row_pieces(bm * U_al, U):
                            eng = nc.vector if asm_i % 2 == 0 else nc.gpsimd
                            eng.tensor_scalar(
                                A2[plo:plo + plen, si,
                                   bn * U_al + boff:bn * U_al + boff + plen],
                                idf[boff:boff + plen, boff:boff + plen],
                                yw[boff:boff + plen, 3 * U + k:3 * U + k + 1],
                                None, op0=OP.mult)
                            asm_i += 1

                # ---- degree + symmetric normalize ----
                dg = ps.tile([128, 512], F32, tag="po0", bufs=2, name="dg")
                for si, (so, ssz) in enumerate(slots):
                    nc.tensor.matmul(dg[:1, :R], ones_c[:ssz, :1],
                                     A2[:ssz, si, :R],
                                     start=(si == 0), stop=(si == 1))
                dinv = sp.tile([1, R], F32, tag="dinv", name="dinv")
                nc.vector.tensor_scalar(dinv[:1, :R], dg[:1, :R], 1e-12, None,
                                        op0=OP.max)
                nc.scalar.activation(dinv[:1, :R], dinv[:1, :R], AF.Sqrt)
                nc.vector.reciprocal(dinv[:1, :R], dinv[:1, :R])
                for si, (so, ssz) in enumerate(slots):
                    op_ = ps.tile([128, 512], F32, tag=f"po{si}", bufs=2,
                                  name=f"op{si}")
                    nc.tensor.matmul(op_[:ssz, :R], dinv[:1, so:so + ssz],
                                     dinv[:1, :R], start=True, stop=True)
                    nc.vector.tensor_mul(A2[:ssz, si, :R],
                                         A2[:ssz, si, :R], op_[:ssz, :R])

                # ---- layer-0 tail: one-time transpose h0T -> h0_rm,
                # then h_rm = P1 * h0_rm ----
                P1 = float(S_SUP * s_l[0] * c1[0])
                transpose_tail(nc, ps, h0T_bf, h0_rm, idb, gtiles, slots, R,
                               1.0, 0)
                for si, (so, ssz) in enumerate(slots):
                    eng = nc.vector if si == 0 else nc.scalar
                    if si == 0:
                        nc.vector.tensor_scalar(h_rm[:ssz, si, :G],
                                                h0_rm[:ssz, si, :G],
                                                P1, None, op0=OP.mult)
                    else:
                        nc.scalar.mul(h_rm[:ssz, si, :G],
                                      h0_rm[:ssz, si, :G], P1)

            # ================= 64 GCNII layers =================
            # Row-major plan: psA (paired banks) holds P_l*hiT for the fp8
            # casts; psO [rs, 500] row-major accumulates S*s_l*(c1*hi + c2*h0
            # + theta*sup@W) via stage1b/c2mm/fp8-DR matmuls; relu writes
            # h_rm directly. No per-layer transposes or psum copies.
            jslices = [(0, 128), (128, R - 128)]
            for l in range(n_layers):
                sl, c1l, c2l = float(s_l[l]), float(c1[l]), float(c2[l])
                cast_s = 1.0 / (sl * c1l)
                c2p = S_SUP * sl * c2l
                if l + 1 < n_layers:
                    out_s = float(S_SUP * s_l[l + 1] * c1[l + 1]) / (S_SUP * sl)
                else:
                    out_s = 1.0 / (S_SUP * sl)   # last layer: unscaled h

                wt8 = wp.tile([128, 4, 2, G], F8, tag="wc", name=f"w{l}")
                nc.sync.dma_start(wt8[:, :, :, :], wq_d[l, :, :])
                nc.gpsimd.tensor_scalar(c2id[:, :], idb[:, :], c2p, None,
                                        op0=OP.mult)

                # stage1a: psA pairs = h_rm.T-contract A2 (feature-major hiT)
                pa_b = [ps.tile([128, 2, 192], F32, tag=f"pa{b}", bufs=2,
                                name=f"pa{l}_{b}") for b in range(2)]
                for g, (go, gsz) in enumerate(gtiles):
                    for si, (so, ssz) in enumerate(slots):
                        nc.tensor.matmul(pa_b[g // 2][:gsz, g % 2, :R],
                                         h_rm[:ssz, si, go:go + gsz],
                                         A2[:ssz, si, :R],
                                         start=(g % 2 == 0 and si == 0),
                                         stop=(g % 2 == 1 and si == 1),
                                         skip_group_check=True)
                # paired fp8 casts: bank01 ACT, bank23 DVE
                nc.scalar.mul(hiT_f8[:, 0:2, :R], pa_b[0][:, :, :R], cast_s)
                nc.vector.tensor_scalar(hiT_f8[:, 2:4, :R], pa_b[1][:, :, :R],
                                        cast_s, None, op0=OP.mult)

                # psO per j-slot, all contributions for slot 0 first so its
                # relu overlaps slot 1's matmuls
                po = [ps.tile([128, 512], F32, tag=f"po{sj}", bufs=2,
                              name=f"po{l}_{sj}") for sj in range(2)]
                for sj, (jo, jsz) in enumerate(jslices):
                    # stage1b: S*s*c1*hi rows via A columns (A symmetric)
                    for ksl, (ko, ksz) in enumerate(slots):
                        nc.tensor.matmul(po[sj][:jsz, :G],
                                         A2[:ksz, ksl, jo:jo + jsz],
                                         h_rm[:ksz, ksl, 0:G],
                                         start=(ksl == 0), stop=False,
                                         skip_group_check=True)
                    # c2 * h0 rows
                    nc.tensor.matmul(po[sj][:jsz, :G], c2id[:jsz, :jsz],
                                     h0_rm[:jsz, sj, 0:G],
                                     start=False, stop=False,
                                     skip_group_check=True)
                    # fp8 DoubleRow: h0 pairs (const) then hi pairs (casts)
                    for p in (2, 3):
                        nc.tensor.matmul(po[sj][:jsz, :G],
                                         h0T_f8[:, 2 * (p - 2):2 * (p - 2) + 2,
                                                jo:jo + jsz],
                                         wt8[:, p, :, :],
                                         start=False, stop=False,
                                         perf_mode=DR, skip_group_check=True)
                    for p in (0, 1):
                        nc.tensor.matmul(po[sj][:jsz, :G],
                                         hiT_f8[:, 2 * p:2 * p + 2, jo:jo + jsz],
                                         wt8[:, p, :, :],
                                         start=False, stop=(p == 1),
                                         perf_mode=DR, skip_group_check=True)
                # relus emitted after ALL h_rm readers of this layer so the
                # WAR dependency keeps the old h visible to stage1a/stage1b
                for sj, (jo, jsz) in enumerate(jslices):
                    if sj == 0:
                        nc.scalar.activation(h_rm[:jsz, sj, :G],
                                             po[sj][:jsz, :G],
                                             AF.Relu, scale=out_s)
                    else:
                        nc.vector.tensor_scalar(h_rm[:jsz, sj, :G],
                                                po[sj][:jsz, :G],
                                                0.0, out_s, op0=OP.max,
                                                op1=OP.mult)

            # ================= head =================
            with tc.tile_pool(name="hd", bufs=1) as hd:
                hT = hd.tile([128, 4, R], BF, tag="hT", name="hT")
                for si, (so, ssz) in enumerate(slots):
                    pt = ps.tile([128, 1024], BF, tag=f"po{si}", bufs=2,
                                 name=f"hdT{si}")
                    for g, (go, gsz) in enumerate(gtiles):
                        nc.tensor.matmul(pt[:gsz, 0:ssz],
                                         h_rm[:ssz, si, go:go + gsz],
                                         idb[:ssz, :ssz], is_transpose=True,
                                         start=True, stop=True,
                                         skip_group_check=True)
                        eng = nc.vector if g % 2 == 0 else nc.scalar
                        if g % 2 == 0:
                            nc.vector.tensor_copy(hT[:gsz, g, so:so + ssz],
                                                  pt[:gsz, 0:ssz])
                        else:
                            nc.scalar.copy(hT[:gsz, g, so:so + ssz],
                                           pt[:gsz, 0:ssz])
                lg = ps.tile([128, 2, 192], F32, tag="pa0", bufs=2, name="lg")
                ki = 0
                for m in range(3):
                    for g, (go, gsz) in enumerate(gtiles):
                        nc.tensor.matmul(lg[:7, 0, :U],
                                         cbf[:gsz, 128 + (m * 4 + g) * 7:
                                             128 + (m * 4 + g) * 7 + 7],
                                         hT[:gsz, g, m * U_al:m * U_al + U],
                                         start=(ki == 0), stop=False,
                                         skip_group_check=True)
                        ki += 1
                nc.tensor.matmul(lg[:7, 0, :U], cbf[0:1, 212:219], ones_r[:1, :U],
                                 start=False, stop=True, skip_group_check=True)
                lgs = hd.tile([7, U], F32, tag="lgs", name="lgs")
                nc.vector.tensor_copy(lgs[:7, :U], lg[:7, 0, :U])
                lt = ps.tile([128, 2, 192], F32, tag="pa1", bufs=2, name="lt")
                nc.tensor.transpose(lt[:U, 0, :7], lgs[:7, :U], idf[:7, :7])
                nmx = hd.tile([U, 1], F32, tag="nmx", name="nmx")
                nc.vector.reduce_max(nmx[:U, :1], lt[:U, 0, :7], AX.X, negate=True)
                esum = hd.tile([U, 1], F32, tag="esum", name="esum")
                edum = hd.tile([U, 7], F32, tag="edum", name="edum")
                nc.scalar.activation(edum[:U, :7], lt[:U, 0, :7], AF.Exp,
                                     bias=nmx[:U, :1], accum_out=esum[:U, :1])
                nls = hd.tile([U, 1], F32, tag="nls", name="nls")
                nc.scalar.activation(nls[:U, :1], esum[:U, :1], AF.Ln)
                nc.vector.tensor_scalar_mul(nls[:U, :1], nls[:U, :1], -1.0)
                osb = hd.tile([U, 7], F32, tag="osb", name="osb")
                nc.vector.tensor_scalar(osb[:U, :7], lt[:U, 0, :7], nmx[:U, :1],
                                        nls[:U, :1], op0=OP.add, op1=OP.add)
                nc.sync.dma_start(out_d[:, :], osb[:U, :7])

    nc.compile()
    return nc


def _prep_shared(inputs, nka, nkv, nkt):
    """Host-side shared (replicated) arrays; returns dict + W scales."""
    Wa, ba = _f32(inputs["Wa"]), _f32(inputs["ba"])
    Wv, bv = _f32(inputs["Wv"]), _f32(inputs["bv"])
    Wt, bt = _f32(inputs["Wt"]), _f32(inputs["bt"])
    spk_emb = _f32(inputs["spk_emb"])
    W_in, b_in = _f32(inputs["W_in"]), _f32(inputs["b_in"])
    W_convs = np.asarray(inputs["W_convs"], np.float64)
    W_fc1, b_fc1 = _f32(inputs["W_fc1"]), _f32(inputs["b_fc1"])
    NSPK = spk_emb.shape[0]

    # projection weights packed [nkf*128, H]: a rows (Wa; ba), v, t (Wt; bt; spk)
    nkf = nka + nkv + nkt
    Wproj = np.zeros((nkf * 128, H), np.float32)
    Wa_aug = np.concatenate([Wa, ba[None, :]], 0)
    Wv_aug = np.concatenate([Wv, bv[None, :]], 0)
    Wt_aug = np.concatenate([Wt, bt[None, :], spk_emb], 0)
    Wproj[:Wa_aug.shape[0]] = Wa_aug
    Wproj[nka * 128:nka * 128 + Wv_aug.shape[0]] = Wv_aug
    Wproj[(nka + nkv) * 128:(nka + nkv) * 128 + Wt_aug.shape[0]] = Wt_aug
    # -> partition-major [128, nkf, H]
    Wproj = Wproj.reshape(nkf, 128, H).transpose(1, 0, 2)

    # W_in packed with b_in as row 300 -> [128, 3, G] partition-major
    Wxp = np.zeros((3 * 128, 512), np.float32)
    Wxp[:H, :G] = W_in
    Wxp[320, :G] = b_in      # ones row lives at a 32-aligned partition
    Wxp = Wxp.reshape(3, 128, 512).transpose(1, 0, 2)

    # fp8 layer weights: s_l * theta_l * W_l packed for DoubleRow
    theta, c1, c2 = _layer_consts()
    tW = theta[:, None, None] * W_convs          # [64, 1000, 500]
    absmax = np.abs(tW).max(axis=(1, 2))
    s_l = 2.0 ** np.floor(np.log2(192.0 / np.maximum(absmax, 1e-30)))
    kcs = _chunks(G, 128)                        # 4 chunks per half
    Wq = np.zeros((NLAYERS, 128, 4 * 2 * G), np.float32)
    for p in range(4):
        for i in range(2):
            ci = 2 * p + i                       # chunk index 0..7
            half, cc = divmod(ci, 4)             # 0: hi rows, 1: h0 rows
            ko, ksz = kcs[cc]
            rows = tW[:, half * G + ko:half * G + ko + ksz, :]  # [64, ksz, 500]
            Wq[:, :ksz, p * 2 * G + i * G:p * 2 * G + (i + 1) * G] = rows
    Wq = np.clip(Wq * s_l[:, None, None], -240.0, 240.0)
    Wq = np.ascontiguousarray(Wq.astype(ml_dtypes.float8_e4m3))

    # bf16 consts [128, 219]: idb | Wf pack [128, 84] | bf1 (row 0, cols 212..)
    cbf = np.zeros((128, 219), np.float32)
    cbf[:, :128] = np.eye(128, dtype=np.float32)
    for m in range(3):
        for g, (go, gsz) in enumerate(kcs):
            cbf[:gsz, 128 + (m * 4 + g) * 7:128 + (m * 4 + g) * 7 + 7] = \
                W_fc1[m * G + go:m * G + go + gsz, :]
    cbf[0, 212:219] = b_fc1

    return {
        "Wproj": _bf(Wproj), "Wxp": _bf(Wxp), "Wq": Wq, "cbf": _bf(cbf),
    }, s_l


def kernel(**inputs):
    global last_results
    inputs = {k: np.asarray(v) for k, v in inputs.items()}
    seq_idx = inputs["seq_idx"].astype(np.int64)
    batch_idx = inputs["batch_idx"].astype(np.int64)
    dia_id = inputs["dia_id"].astype(np.int64)
    fea_a, fea_v, fea_t = inputs["fea_a"], inputs["fea_v"], inputs["fea_t"]
    speaker = inputs["speaker"]
    spk_emb = inputs["spk_emb"]
    N = seq_idx.shape[0]
    NSPK = spk_emb.shape[0]
    da, dv, dt = fea_a.shape[2], fea_v.shape[2], fea_t.shape[2]

    # ---- shard dialogues over cores ----
    uniq, counts = np.unique(dia_id, return_counts=True)
    bins, loads = _lpt_assign(counts, NCORES)
    U = max(int(loads.max()), 44)   # R = 3U must exceed 128
    positions = {int(d): np.where(dia_id == d)[0] for d in uniq}
    core_utts = []
    for b in range(NCORES):
        if bins[b]:
            idx = np.sort(np.concatenate([positions[d] for d in bins[b]]))
        else:
            idx = np.zeros(0, np.int64)
        core_utts.append(idx.astype(np.int64))

    nka = _pad128(da + 1) // 128
    nkv = _pad128(dv + 1) // 128
    nkt = _pad128(dt + 1 + NSPK) // 128
    nkf = nka + nkv + nkt

    spk = np.argmax(_f32(speaker)[seq_idx, batch_idx], axis=-1)
    shared, s_l = _prep_shared(inputs, nka, nkv, nkt)
    build_kernel._w_scales = s_l

    in_maps = []
    for b in range(NCORES):
        utts = core_utts[b]
        nreal = len(utts)
        f_all = np.zeros((nkf * 128, U), np.float32)
        cf = np.zeros((128, U + 128), np.float32)
        cf[:, U:U + 128] = np.eye(128, dtype=np.float32)
        if nreal:
            f_all[:da, :nreal] = _f32(fea_a)[seq_idx[utts], batch_idx[utts]].T
            f_all[da, :nreal] = 1.0
            vo = nka * 128
            f_all[vo:vo + dv, :nreal] = _f32(fea_v)[seq_idx[utts], batch_idx[utts]].T
            f_all[vo + dv, :nreal] = 1.0
            to = (nka + nkv) * 128
            f_all[to:to + dt, :nreal] = _f32(fea_t)[seq_idx[utts], batch_idx[utts]].T
            f_all[to + dt, :nreal] = 1.0
            oh = np.zeros((NSPK, nreal), np.float32)
            oh[spk[utts], np.arange(nreal)] = 1.0
            f_all[to + dt + 1:to + dt + 1 + NSPK, :nreal] = oh
            dd = dia_id[utts]
            cf[:nreal, :nreal] = (dd[:, None] == dd[None, :]).astype(np.float32)
        f_pack = f_all.reshape(nkf, 128, U).transpose(1, 0, 2)
        in_maps.append({"f_all": _bf(f_pack), "cf32": cf, **shared})

    key = (U, nka, nkv, nkt) + tuple(float(s) for s in s_l)
    if key not in _BUILD_CACHE:
        _BUILD_CACHE[key] = build_kernel(U, nka, nkv, nkt)
    nc = _BUILD_CACHE[key]

    trace = bool(int(os.environ.get("BASS_GCN_TRACE", "0")))
    res = run_bass_kernel_spmd(nc, in_maps, core_ids=list(range(NCORES)),
                               trace=trace)
    last_results = res

    out_full = np.zeros((N, 7), np.float32)
    for b in range(NCORES):
        utts = core_utts[b]
        if len(utts):
            out_full[utts] = np.asarray(res.results[b]["out"], np.float32)[:len(utts)]
    return out_full
